# revision 30
# baseline (speedup 1.0000x reference)
"""Trainium2 Bass kernel for nn_CellEncoder (2-layer GraphSAGE, mean aggregation).

Strategy (8 NeuronCores, SPMD, node-partitioned), v5:
  - Core c owns nodes [c*npc, (c+1)*npc).  Aggregation is linear, so the
    dense transform is applied FIRST: z = h @ W_l.T reduces gather width
    from in_dim (1000) to emb (128) values per edge.  All tables/operands
    are bf16 (PSUM accumulation fp32); tolerance is 2e-2, bf16 ~5e-3.
  - The z table is split into SEGS=3 row segments, each AllGathered
    separately as soon as its producer rows are ready: segment-0 gathers
    start while phase A is still computing later segments' z, and the
    layer-1 boundary only stalls on the (small) last segment's AllGather.
  - Edges grouped by (dst tile, src segment); slots packed into 128-slot
    chunks.  One dma_gather per (tile, segment) (<= 8 chunks, 1024 idxs,
    single_packet fast path) on the least-chunk-loaded of the 4 SWDGE
    queues (a plain round-robin aliases with piece sizes and starves
    queues).  Padding slots are NEGATIVE indices at the gather tail (the
    DMA skips them) and num_idxs_reg is reg_load-ed from a per-core count
    table, so the descriptor stream is exactly the core's edge count.
  - The int16 index slab (shared by both layers -- same edges) is loaded
    once into SBUF.  Gather buffers are fixed SBUF slabs with manual
    cycling per segment stream; earlier segments get deeper lookahead.
  - One-hot scatter matrices S[e,d] = (dst(e)==d) built on DVE per tile
    with a batched is_equal against a materialized iota (contiguous in1).
    PE accumulates aggT[f,d] += G_chunk.T @ S_chunk in PSUM per group of
    4 tiles.  Skipped padding slots hold zeros/stale rows (finite), S=0.
  - Phase A computes z0 feature-major with stationary W_l0, transposes
    per tile on the PE against identity, and only then computes the r0
    term -- z (which gates the AllGathers) finishes as early as possible.
  - ELU's "-1" is folded out: the device computes h~ = elu(x)+1; the next
    layer's bias is adjusted on the host and the host subtracts 1 from
    the final output (requires min in-degree >= 1, checked on host).
    exp(min(x,0)) is computed as Exp(-Relu(-x)) on the scalar engine.

kernel(**inputs) takes FULL inputs, shards internally, runs one NEFF on
cores 0-7 via bass_utils.run_bass_kernel_spmd, returns the full output.
"""
import os
import sys

import numpy as np

for _p in ("/opt/trn_rl_repo", "/root/.axon_site/_ro/trn_rl_repo"):
    if os.path.isdir(_p) and _p not in sys.path:
        sys.path.append(_p)

import ml_dtypes

import concourse.bass as bass
import concourse.bacc as bacc
import concourse.mybir as mybir
import concourse.tile as tile
from concourse import bass_utils

P = 128
F32 = mybir.dt.float32
BF16 = mybir.dt.bfloat16
I16 = mybir.dt.int16
I32 = mybir.dt.int32
AF = mybir.ActivationFunctionType
ALU = mybir.AluOpType

SCRATCH = 49152    # SWDGE ring: scratch//16 descriptors per queue
PIECE = 8          # max chunks per gather (1024 idxs, single_packet path)
SEGS = 3
NBUFS = (15, 8, 7)     # per-segment gather tile buffers in flight
LOOKS = (14, 6, 4)     # per-segment issue leads (tiles); < NBUFS[k]
L1_GATES = (28, 40, None)  # layer-0 tile from which layer-1 seg-k issues


def build_meta(N, NC, dst, src, tiles_per_st):
    """Static chunk structure (shared across cores; max-over-core sizes),
    per-core gather-index / dst-id slabs with -1 padding tails, and per-core
    per-gather valid-index counts."""
    npc = N // NC
    half = npc // 2
    TPC = (npc + P - 1) // P
    NPAD = TPC * P
    NST = (TPC + tiles_per_st - 1) // tiles_per_st

    b1 = (npc + SEGS - 1) // SEGS
    segb = [min(k * b1, npc) for k in range(SEGS + 1)]
    segsz = [segb[k + 1] - segb[k] for k in range(SEGS)]
    assert all(NC * s <= 32768 for s in segsz)

    c = dst // npc
    d = (dst - c * npc).astype(np.int64)
    t = d // P
    did = d % P
    sc = src // npc
    sp = src - sc * npc
    tb = np.minimum(sp // b1, SEGS - 1).astype(np.int64)
    row = sc * np.array(segsz)[tb] + (sp - np.array(segb)[tb])
    assert row.max() < 32768

    nseg = np.zeros((NC, TPC, SEGS), np.int64)
    np.add.at(nseg, (c, t, tb), 1)
    KS = np.maximum(1, (nseg.max(axis=0) + P - 1) // P)  # [TPC, SEGS]
    assert KS.max() <= PIECE, KS.max()

    Ktot = KS.sum(axis=1)
    chunk_base = np.concatenate([[0], np.cumsum(Ktot)])
    NCHUNK = int(chunk_base[-1])
    KMAXS = [int(KS[:, k].max()) for k in range(SEGS)]

    st_tiles = [list(range(s * tiles_per_st, min((s + 1) * tiles_per_st, TPC)))
                for s in range(NST)]

    # one gather per (tile, segment)
    idx_off16 = np.zeros((TPC, SEGS), np.int64)
    gidx = np.zeros((TPC, SEGS), np.int64)
    off = 0
    g = 0
    for tt in range(TPC):
        for k in range(SEGS):
            idx_off16[tt, k] = off
            gidx[tt, k] = g
            off += int(KS[tt, k]) * P // 16
            g += 1
    NIDX16 = int(off)
    NG = int(g)

    idx_slab = np.zeros((NC, P, NIDX16), np.int16)
    dstid_slab = np.full((NC, P, NCHUNK), -1.0, np.float32)
    counts = np.zeros((NC, NG), np.int32)
    cnt = np.zeros((NC, NPAD), np.int64)

    order = np.lexsort((tb, t, c))
    co, to, tbo = c[order], t[order], tb[order]
    rowo, dido, do_ = row[order], did[order], d[order]
    np.add.at(cnt, (co, do_), 1)

    key = (co * TPC + to) * SEGS + tbo
    bounds = np.concatenate([[0], np.nonzero(np.diff(key))[0] + 1, [len(key)]])
    filled = np.zeros((NC, TPC, SEGS), bool)
    for bi in range(len(bounds) - 1):
        lo_, hi_ = int(bounds[bi]), int(bounds[bi + 1])
        if lo_ == hi_:
            continue
        cc, tt, kk = int(co[lo_]), int(to[lo_]), int(tbo[lo_])
        n = hi_ - lo_
        K = int(KS[tt, kk])
        v = np.full(K * P, -1, np.int16)
        v[:n] = rowo[lo_:hi_]
        counts[cc, gidx[tt, kk]] = n
        filled[cc, tt, kk] = True
        o16 = int(idx_off16[tt, kk])
        w = v.reshape(K * P // 16, 16).T
        idx_slab[cc, :, o16:o16 + K * P // 16] = np.tile(w, (8, 1))
        ch0 = int(chunk_base[tt]) + int(KS[tt, :kk].sum())
        local = np.arange(n)
        dstid_slab[cc, local % P, ch0 + local // P] = dido[lo_:hi_]

    # empty (core,tile,seg): one dummy valid idx (row 0; dstid -1 -> S = 0)
    for cc, tt, kk in zip(*np.nonzero(~filled)):
        K = int(KS[tt, kk])
        v = np.full(K * P, -1, np.int16)
        v[0] = 0
        counts[cc, gidx[tt, kk]] = 1
        o16 = int(idx_off16[tt, kk])
        w = v.reshape(K * P // 16, 16).T
        idx_slab[cc, :, o16:o16 + K * P // 16] = np.tile(w, (8, 1))

    inv = (1.0 / np.maximum(cnt, 1)).astype(np.float32)
    shift_ok = bool(cnt[:, :npc].min() >= 1)

    return dict(
        npc=npc, half=half, TPC=TPC, NPAD=NPAD, NST=NST, st_tiles=st_tiles,
        segb=segb, segsz=segsz,
        KS=KS.astype(int), KMAXS=KMAXS,
        chunk_base=[int(v) for v in chunk_base], NCHUNK=NCHUNK,
        idx_off16=idx_off16.astype(int), gidx=gidx.astype(int),
        NIDX16=NIDX16, NG=NG,
        idx_slab=idx_slab, dstid_slab=dstid_slab, counts=counts, inv=inv,
        shift_ok=shift_ok,
    )


# ---------------------------------------------------------------------------
# device kernel builder
# ---------------------------------------------------------------------------

def build_kernel(meta, in_dim, NC):
    npc = meta["npc"]
    TPC, NPAD, NST = meta["TPC"], meta["NPAD"], meta["NST"]
    NCHUNK, NIDX16, NG = meta["NCHUNK"], meta["NIDX16"], meta["NG"]
    KS, KMAXS = meta["KS"], meta["KMAXS"]
    segb, segsz = meta["segb"], meta["segsz"]
    chunk_base = meta["chunk_base"]
    idx_off16, gidx = meta["idx_off16"], meta["gidx"]
    shift = meta["shift_ok"]
    GC = (in_dim + P - 1) // P
    GPAD = GC * P
    WMAX = max(len(ts) for ts in meta["st_tiles"]) * P
    KT2MAX = int(max(KS[t].sum() for t in range(TPC)))
    qload = [0, 0, 0, 0]

    nc = bacc.Bacc("TRN2", target_bir_lowering=False, debug=False,
                   enable_asserts=False, num_devices=NC,
                   dynamic_dma_scratch_size=SCRATCH, num_swdge_queues=4)

    x_d = nc.dram_tensor("x_pad", [GPAD, NPAD], BF16, kind="ExternalInput").ap()
    w0l_d = nc.dram_tensor("W0lT", [GPAD, P], BF16, kind="ExternalInput").ap()
    w0r_d = nc.dram_tensor("W0rT", [GPAD, P], BF16, kind="ExternalInput").ap()
    w1l_d = nc.dram_tensor("W1lT", [P, P], BF16, kind="ExternalInput").ap()
    w1r_d = nc.dram_tensor("W1rT", [P, P], BF16, kind="ExternalInput").ap()
    b0_d = nc.dram_tensor("b0col", [P, 1], F32, kind="ExternalInput").ap()
    b1_d = nc.dram_tensor("b1col", [P, 1], F32, kind="ExternalInput").ap()
    inv_d = nc.dram_tensor("invt", [P, NPAD], BF16, kind="ExternalInput").ap()
    idx_d = nc.dram_tensor("idx16", [P, NIDX16], I16, kind="ExternalInput").ap()
    cnts_d = nc.dram_tensor("gcnts", [P, NG], I32, kind="ExternalInput").ap()
    iota_d = nc.dram_tensor("iota", [P, P], BF16, kind="ExternalInput").ap()
    ident_d = nc.dram_tensor("ident", [P, P], BF16, kind="ExternalInput").ap()
    dst_d = nc.dram_tensor("dstid", [P, NCHUNK], BF16, kind="ExternalInput").ap()
    out_d = nc.dram_tensor("outT", [P, NPAD], BF16, kind="ExternalOutput").ap()

    with tile.TileContext(nc, num_cores=NC) as tc:
        with (
            tc.tile_pool(name="const", bufs=1) as cpool,
            tc.tile_pool(name="slab", bufs=1) as slab,
            tc.tile_pool(name="zp", bufs=4) as zpool,
            tc.tile_pool(name="ep", bufs=2) as epool,
            tc.tile_pool(name="sp", bufs=3) as spool,
            tc.tile_pool(name="pz", bufs=3, space="PSUM") as pz,
            tc.tile_pool(name="pr", bufs=2, space="PSUM") as pr,
            tc.tile_pool(name="dram", bufs=1, space="DRAM") as dram,
        ):
            # ---- constants ----
            w0l_sb = cpool.tile([P, GC * P], BF16)
            w0r_sb = cpool.tile([P, GC * P], BF16)
            for gc in range(GC):
                nc.sync.dma_start(out=w0l_sb[:, gc * P:(gc + 1) * P],
                                  in_=w0l_d[gc * P:(gc + 1) * P, :])
                nc.sync.dma_start(out=w0r_sb[:, gc * P:(gc + 1) * P],
                                  in_=w0r_d[gc * P:(gc + 1) * P, :])
            w1l_sb = cpool.tile([P, P], BF16)
            nc.sync.dma_start(out=w1l_sb[:], in_=w1l_d[:])
            w1r_sb = cpool.tile([P, P], BF16)
            nc.sync.dma_start(out=w1r_sb[:], in_=w1r_d[:])
            b0_sb = cpool.tile([P, 1], F32)
            nc.sync.dma_start(out=b0_sb[:], in_=b0_d[:])
            b1_sb = cpool.tile([P, 1], F32)
            nc.sync.dma_start(out=b1_sb[:], in_=b1_d[:])
            mone_sb = cpool.tile([P, 1], BF16)
            nc.vector.memset(mone_sb[:], -1.0)
            # iota materialized at full S width: contiguous is_equal operand
            iota_sb = cpool.tile([P, KT2MAX * P], BF16)
            ident_sb = cpool.tile([P, P], BF16)
            nc.sync.dma_start(out=ident_sb[:], in_=ident_d[:])
            dst_sb = cpool.tile([P, NCHUNK], BF16)
            idx_sb = cpool.tile([P, NIDX16], I16)
            nc.scalar.dma_start(out=idx_sb[:], in_=idx_d[:])
            cnts_sb = cpool.tile([P, NG], I32)
            nc.scalar.dma_start(out=cnts_sb[:], in_=cnts_d[:])

            rb0_sb = slab.tile([P, NPAD], BF16)
            rb1_sb = slab.tile([P, NPAD], BF16)

            # gather slabs (manual buffer cycling); zeroed once on the (idle
            # through phase A) Pool engine so skipped padding slots never
            # feed NaN/Inf garbage into the PE
            gseg_sb = []
            for k in range(SEGS):
                # explicit tags: same-line tile() calls share an auto-tag,
                # which in a bufs=1 pool would serialize the slabs' lifetimes
                t_ = cpool.tile([P, NBUFS[k] * KMAXS[k] * P], BF16,
                                tag=f"gseg{k}")
                nc.gpsimd.memset(t_[:], 0.0)
                gseg_sb.append(t_)

            greg = nc.gpsimd.alloc_register("gcnt")

            # ---- collective buffers ----
            def cc_set(nm):
                ins_ = [dram.tile([segsz[k], P], BF16, name=f"cci{k}_{nm}",
                                  tag=f"cci{k}_{nm}")
                        for k in range(SEGS)]
                outs_ = [dram.tile([NC * segsz[k], P], BF16, addr_space="Shared",
                                   name=f"cco{k}_{nm}", tag=f"cco{k}_{nm}")
                         for k in range(SEGS)]
                return ins_, outs_

            cc0i, cc0o = cc_set("0")
            cc1i, cc1o = cc_set("1")
            rg = [list(range(NC))]

            def z_to_cc(z_sb, tt, cci):
                r0, r1 = tt * P, min(tt * P + P, npc)
                for k in range(SEGS):
                    lo_s, hi_s = max(r0, segb[k]), min(r1, segb[k + 1])
                    if hi_s > lo_s:
                        nc.scalar.dma_start(
                            out=cci[k][lo_s - segb[k]:hi_s - segb[k], :],
                            in_=z_sb[lo_s - r0:hi_s - r0, :])

            def ag(cci, cco, k):
                nc.gpsimd.collective_compute(
                    "AllGather", ALU.bypass, replica_groups=rg,
                    ins=[cci[k][:].opt()], outs=[cco[k][:].opt()])

            # ---- phase A: z0 (fm + PE transpose) first, then r0 ----
            pa_groups = [list(range(s * 2, min(s * 2 + 2, TPC)))
                         for s in range((TPC + 1) // 2)]
            # AG for segment k fires once all z rows < segb[k+1] are written
            ag_after = {}
            for k in range(SEGS - 1):
                g_ = next(i for i, gts in enumerate(pa_groups)
                          if (gts[-1] + 1) * P >= segb[k + 1])
                ag_after.setdefault(g_, []).append(k)
            with (
                tc.tile_pool(name="xp", bufs=2) as xpool,
                tc.tile_pool(name="pf", bufs=2, space="PSUM") as pf,
            ):
                for s, ts in enumerate(pa_groups):
                    w = len(ts) * P
                    c0 = ts[0] * P
                    xg = xpool.tile([P, GC * w], BF16, tag="xg",
                                    padded_shape=[P, GC * 2 * P])
                    nc.sync.dma_start(
                        out=xg[:].rearrange("p (gc j) -> p gc j", gc=GC),
                        in_=x_d[:, c0:c0 + w].rearrange("(gc p) j -> p gc j", p=P))
                    zfm = pf.tile([P, w], F32, tag="zfm", padded_shape=[P, 2 * P])
                    for gc in range(GC):
                        nc.tensor.matmul(out=zfm[:],
                                         lhsT=w0l_sb[:, gc * P:(gc + 1) * P],
                                         rhs=xg[:, gc * w:(gc + 1) * w],
                                         start=(gc == 0), stop=(gc == GC - 1))
                    zfm_sb = zpool.tile([P, w], BF16, tag="zfm_sb",
                                        padded_shape=[P, 2 * P])
                    nc.vector.tensor_copy(out=zfm_sb[:], in_=zfm[:])
                    for ti, tt in enumerate(ts):
                        zT = pz.tile([P, P], F32, tag="zps")
                        nc.tensor.matmul(out=zT[:],
                                         lhsT=zfm_sb[:, ti * P:(ti + 1) * P],
                                         rhs=ident_sb[:], start=True, stop=True)
                        z0sb = zpool.tile([P, P], BF16, tag="zsb")
                        nc.vector.tensor_copy(out=z0sb[:], in_=zT[:])
                        z_to_cc(z0sb, tt, cc0i)
                    r0ps = pr.tile([P, w], F32, tag="rps", padded_shape=[P, WMAX])
                    for gc in range(GC):
                        nc.tensor.matmul(out=r0ps[:],
                                         lhsT=w0r_sb[:, gc * P:(gc + 1) * P],
                                         rhs=xg[:, gc * w:(gc + 1) * w],
                                         start=(gc == 0), stop=(gc == GC - 1))
                    nc.vector.tensor_tensor(out=rb0_sb[:, c0:c0 + w], in0=r0ps[:],
                                            in1=b0_sb[:, :1].to_broadcast([P, w]),
                                            op=ALU.add)
                    for k in ag_after.get(s, []):
                        ag(cc0i, cc0o, k)
            ag(cc0i, cc0o, SEGS - 1)
            # S-build constants, deferred so phase A's z writes own the
            # scalar queue (first S build happens well after these land)
            for k in range(KT2MAX):
                nc.scalar.dma_start(out=iota_sb[:, k * P:(k + 1) * P],
                                    in_=iota_d[:])
            nc.scalar.dma_start(out=dst_sb[:], in_=dst_d[:])
            invall_sb = cpool.tile([P, NPAD], BF16)
            nc.scalar.dma_start(out=invall_sb[:], in_=inv_d[:])

            # pa opens after phase A's pf pool closes (PSUM: 8 banks, 7+7)
            pa_cm = tc.tile_pool(name="pa", bufs=2, space="PSUM")
            pa = pa_cm.__enter__()

            # ---- aggregation machinery -------------------------------------
            tabs = (cc0o, cc1o)
            state = [0] * SEGS  # issue cursor per segment stream, in steps

            def issue_gather(k, step):
                layer, tt = divmod(step, TPC)
                buf = step % NBUFS[k]
                base = buf * KMAXS[k] * P
                K = int(KS[tt, k])
                g = int(gidx[tt, k])
                o16 = int(idx_off16[tt, k])
                q = min(range(4), key=lambda i: qload[i])
                qload[q] += K
                nc.gpsimd.reg_load(greg, cnts_sb[0:1, g:g + 1])
                nc.gpsimd.dma_gather(
                    out_ap=gseg_sb[k][:, base:base + K * P]
                    .rearrange("p (kk e) -> p kk e", e=P),
                    in_ap=tabs[layer][k][:],
                    idxs_ap=idx_sb[:, o16:o16 + K * 8],
                    num_idxs=K * P, num_idxs_reg=greg, elem_size=P,
                    single_packet=True, queue_num=q)

            def pump(k, limit):
                while state[k] < min(limit, 2 * TPC):
                    issue_gather(k, state[k])
                    state[k] += 1

            def aggregate_tile(layer, tt, aggps, ti):
                nch = int(KS[tt].sum())
                cb0 = chunk_base[tt]
                s_sb = spool.tile([P, nch * P], BF16, tag="ssb",
                                  padded_shape=[P, KT2MAX * P])
                nc.vector.tensor_tensor(
                    out=s_sb[:].rearrange("p (n e) -> p n e", e=P),
                    in0=dst_sb[:, cb0:cb0 + nch].unsqueeze(2)
                    .to_broadcast([P, nch, P]),
                    in1=iota_sb[:, :nch * P].rearrange("p (n e) -> p n e", e=P),
                    op=ALU.is_equal)
                step = layer * TPC + tt
                j = 0
                for k in range(SEGS):
                    base = (step % NBUFS[k]) * KMAXS[k] * P
                    for jj in range(int(KS[tt, k])):
                        g_ap = gseg_sb[k][:, base + jj * P:base + (jj + 1) * P]
                        nc.tensor.matmul(out=aggps[:, ti * P:(ti + 1) * P],
                                         lhsT=g_ap,
                                         rhs=s_sb[:, j * P:(j + 1) * P],
                                         start=(j == 0), stop=(j == nch - 1))
                        j += 1

            s_ag = ((npc // 2 + P - 1) // P - 1) // len(meta["st_tiles"][0])
            # layer-1 AG trigger groups: z1 rows < segb[k+1] complete
            ag1_after = {}
            for k in range(SEGS - 1):
                g_ = next(i for i, gts in enumerate(meta["st_tiles"])
                          if (gts[-1] + 1) * P >= segb[k + 1])
                ag1_after.setdefault(g_, []).append(k)

            def run_layer(layer, rb_slab, out_cb, mid_cb=None):
                for s, ts in enumerate(meta["st_tiles"]):
                    w = len(ts) * P
                    c0 = ts[0] * P
                    aggps = pa.tile([P, w], F32, tag="aggps",
                                    padded_shape=[P, WMAX])
                    for ti, tt in enumerate(ts):
                        step = layer * TPC + tt
                        for k in reversed(range(SEGS)):
                            limit = step + LOOKS[k] + 1
                            if layer == 0:
                                if L1_GATES[k] is None:
                                    # its AG trigger comes after this loop
                                    limit = min(limit, TPC)
                                elif tt < L1_GATES[k]:
                                    limit = min(limit, TPC)
                            pump(k, limit)
                        aggregate_tile(layer, tt, aggps, ti)
                    x2 = epool.tile([P, w], BF16, tag="x2", padded_shape=[P, WMAX])
                    nc.vector.tensor_tensor(out=x2[:], in0=aggps[:],
                                            in1=invall_sb[:, c0:c0 + w],
                                            op=ALU.mult)
                    x3 = epool.tile([P, w], BF16, tag="x3", padded_shape=[P, WMAX])
                    nc.vector.tensor_tensor(out=x3[:], in0=x2[:],
                                            in1=rb_slab[:, c0:c0 + w], op=ALU.add)
                    xm = epool.tile([P, w], BF16, tag="xm", padded_shape=[P, WMAX])
                    nc.scalar.activation(out=xm[:], in_=x3[:], func=AF.Relu)
                    # exp(min(x,0)) = Exp(-Relu(-x)), both on the scalar engine
                    xc = epool.tile([P, w], BF16, tag="xc", padded_shape=[P, WMAX])
                    nc.scalar.activation(out=xc[:], in_=x3[:], func=AF.Relu,
                                         scale=-1.0)
                    xe = epool.tile([P, w], BF16, tag="xe", padded_shape=[P, WMAX])
                    nc.scalar.activation(out=xe[:], in_=xc[:], func=AF.Exp,
                                         scale=-1.0)
                    h = epool.tile([P, w], BF16, tag="h", padded_shape=[P, WMAX])
                    nc.vector.tensor_tensor(out=h[:], in0=xm[:], in1=xe[:],
                                            op=ALU.add)
                    if not shift:
                        h2 = epool.tile([P, w], BF16, tag="h2",
                                        padded_shape=[P, WMAX])
                        nc.vector.tensor_tensor(
                            out=h2[:], in0=h[:],
                            in1=mone_sb[:, :1].to_broadcast([P, w]), op=ALU.add)
                        h = h2
                    out_cb(s, ts, w, c0, h)
                    if mid_cb is not None:
                        mid_cb(s)

            # ---- layer 0 aggregate -> h1T -> z1/rb1T ----
            def l0_out(s, ts, w, c0, h):
                for ti, tt in enumerate(ts):
                    z1ps = pz.tile([P, P], F32, tag="zps")
                    nc.tensor.matmul(out=z1ps[:],
                                     lhsT=h[:, ti * P:(ti + 1) * P],
                                     rhs=w1l_sb[:], start=True, stop=True)
                    z1sb = zpool.tile([P, P], BF16, tag="zsb")
                    nc.vector.tensor_copy(out=z1sb[:], in_=z1ps[:])
                    z_to_cc(z1sb, tt, cc1i)
                r1ps = pr.tile([P, w], F32, tag="rps", padded_shape=[P, WMAX])
                nc.tensor.matmul(out=r1ps[:], lhsT=w1r_sb[:], rhs=h[:],
                                 start=True, stop=True)
                nc.vector.tensor_tensor(out=rb1_sb[:, c0:c0 + w], in0=r1ps[:],
                                        in1=b1_sb[:, :1].to_broadcast([P, w]),
                                        op=ALU.add)

            # prologue: deepest stream first, then shorter leads
            pump(0, NBUFS[0])
            for k in range(1, SEGS):
                pump(k, LOOKS[k])

            def l0_mid(s):
                for k in ag1_after.get(s, []):
                    ag(cc1i, cc1o, k)

            run_layer(0, rb0_sb, l0_out, mid_cb=l0_mid)
            ag(cc1i, cc1o, SEGS - 1)

            # ---- layer 1 aggregate -> output ----
            def l1_out(s, ts, w, c0, h):
                nc.scalar.dma_start(out=out_d[:, c0:c0 + w], in_=h[:])

            run_layer(1, rb1_sb, l1_out)
            pa_cm.__exit__(None, None, None)

    nc.compile()
    return nc


# ---------------------------------------------------------------------------
# entry point
# ---------------------------------------------------------------------------

def _bf16(a):
    return np.asarray(a, np.float32).astype(ml_dtypes.bfloat16)


def _prepare(x, knn_edge_index, W_l0, b_l0, W_r0, W_l1, b_l1, W_r1,
             NC=8, tiles_per_st=4):
    x = np.asarray(x, np.float32)
    e = np.asarray(knn_edge_index)
    in_dim, N = x.shape
    src, dst = e[0].astype(np.int64), e[1].astype(np.int64)
    meta = build_meta(N, NC, dst, src, tiles_per_st)
    npc, NPAD = meta["npc"], meta["NPAD"]
    GC = (in_dim + P - 1) // P
    GPAD = GC * P

    W_l1 = np.asarray(W_l1, np.float32)
    W_r1 = np.asarray(W_r1, np.float32)
    b1 = np.asarray(b_l1, np.float32)
    if meta["shift_ok"]:
        b1 = b1 - W_l1.sum(axis=1) - W_r1.sum(axis=1)

    w0l = np.zeros((GPAD, P), np.float32); w0l[:in_dim] = np.asarray(W_l0).T
    w0r = np.zeros((GPAD, P), np.float32); w0r[:in_dim] = np.asarray(W_r0).T
    shared = {
        "W0lT": _bf16(w0l), "W0rT": _bf16(w0r),
        "W1lT": _bf16(np.ascontiguousarray(W_l1.T)),
        "W1rT": _bf16(np.ascontiguousarray(W_r1.T)),
        "b0col": np.asarray(b_l0, np.float32).reshape(P, 1),
        "b1col": b1.reshape(P, 1),
        "iota": _bf16(np.broadcast_to(np.arange(P, dtype=np.float32), (P, P))),
        "ident": _bf16(np.eye(P, dtype=np.float32)),
    }
    in_maps = []
    for c in range(NC):
        xp = np.zeros((GPAD, NPAD), np.float32)
        xp[:in_dim, :npc] = x[:, c * npc:(c + 1) * npc]
        m = dict(shared)
        m["x_pad"] = _bf16(xp)
        m["invt"] = _bf16(np.broadcast_to(meta["inv"][c], (P, NPAD)))
        m["idx16"] = np.ascontiguousarray(meta["idx_slab"][c])
        m["gcnts"] = np.ascontiguousarray(
            np.broadcast_to(meta["counts"][c], (P, meta["NG"])).astype(np.int32))
        m["dstid"] = _bf16(meta["dstid_slab"][c])
        in_maps.append(m)
    return meta, in_dim, in_maps


def run(inputs, NC=8, tiles_per_st=4, trace=False, **run_kwargs):
    meta, in_dim, in_maps = _prepare(**inputs, NC=NC, tiles_per_st=tiles_per_st)
    nc = build_kernel(meta, in_dim, NC)
    res = bass_utils.run_bass_kernel_spmd(
        nc, in_maps, core_ids=list(range(NC)), trace=trace, **run_kwargs)
    npc = meta["npc"]
    sub = 1.0 if meta["shift_ok"] else 0.0
    out = np.concatenate(
        [res.results[c]["outT"][:, :npc].T.astype(np.float32) - sub
         for c in range(NC)], axis=0)
    return np.ascontiguousarray(out), res


def kernel(**inputs) -> np.ndarray:
    out, _ = run(inputs)
    return out
